# revision 37
# baseline (speedup 1.0000x reference)
"""Trainium2 Bass kernel for CNN+GCN+MLP (nn_CNNGCN_18236431139458).

Strategy (8 NeuronCores, one chip):
  - Conv + both GCN layers: data-parallel over batch (4 samples/core).
    The scatter-aggregate is a dense matmul against the normalized
    adjacency A^T (built host-side from edge_index), held RESIDENT in
    SBUF in fp8 and shared by BOTH GCN layers via DoubleRow (2x PE
    rate, 2 k-chunks per instruction -> 4x MACs/cycle vs bf16).
  - Conv also runs in fp8 DoubleRow: the 3 taps are padded to 4 and
    packed 2-per-instruction; x is duplicated host-side at offsets
    {0,1} so each instruction reads a [ic, 2, n] window.
  - Precision plan (validated vs the fp32 reference in numpy):
    weights for the GCN linears stay bf16 (weight quantization error is
    coherent across nodes and is NOT averaged away by aggregation);
    activations + A in fp8 (their noise is independent per edge/node
    and averages down ~sqrt(deg) in each aggregation). Scales are
    folded into weights host-side (conv_w*32, gW1/32, gW2*16, W1/16)
    so every PSUM->SBUF fixup is a single add+relu op.
  - MLP: W1 (262144 x 100) is sharded over rows (nodes) across cores
    and held resident in SBUF. An on-device AllToAll reshards the GCN
    output from batch-sharded to node-sharded; each core computes a
    partial z^T [100, 32] (moving dim = 32 samples, 3x fewer PE cycles
    than the [32,100] orientation); an AllReduce sums partials and
    every core computes the full [32] output locally. z columns are in
    (s_local, core) order; kernel() un-permutes on the host.
  - DMA queues: SP HWDGE carries A^T + A2A staging, Act HWDGE carries
    x/small weights + A2A returns, gpsimd SWDGE streams W1 — one
    serial queue would otherwise dominate the wall clock (~565 ns
    dispatch per DMA plus serialized transfers).

Layouts (per core):
  xT2  [128 ic, 4 s, 2 j, 2052]  x duplicated at row offsets {0,1}
  h*T  [128 f, 4 s, 2048 n]      feature-major activations
  hw*  [128 n, 16 nch, 4*128]    node-major GCN linear outputs (fp8)
  at8  [128 p, 16 sc, 2048 dst]  = A^T[sc*128+p, dst] fp8, resident
  w1   [128 f, 256 n, 100]       W1 row-shard, resident
  h2a  [128 f, 4 s, 8 core, 256] post-A2A, (s, core)-ordered samples
"""

import numpy as np
import ml_dtypes

import concourse.bass as bass
import concourse.mybir as mybir
import concourse.tile as tile
from concourse.tile import add_dep_helper
from concourse import bacc
from concourse.bass_utils import run_bass_kernel_spmd

BF16 = mybir.dt.bfloat16
FP8 = mybir.dt.float8e4
F32 = mybir.dt.float32
NP_BF16 = ml_dtypes.bfloat16
NP_FP8 = mybir.dt.np(FP8)

B, H, E = 32, 2050, 128
N = 2048
C = 128
G1 = G2 = 128
MLPD = 100
KS = 3
NE = 32768
NCORES = 8
BL = B // NCORES          # 4 samples per core
NSH = N // NCORES         # 256 nodes per core (W1 row shard)
HP = 2052                 # padded x height (window reads up to row 2050)
RG = [list(range(NCORES))]

Relu = mybir.ActivationFunctionType.Relu
DoubleRow = mybir.MatmulPerfMode.DoubleRow
ALU_ADD = mybir.AluOpType.add
ALU_MAX = mybir.AluOpType.max


def _emit_front(nc, tc, pools, tensors, n_warm=16):
    """conv + GCN1 + GCN2 -> list of per-sample h2T [128, 8, 256] bf16."""
    acts, psum = pools["acts"], pools["psum"]
    xT2_sb = tensors["xT2_sb"]
    wc_sb = tensors["wc_sb"]
    at8_sb = tensors["at8_sb"]

    # ---- PE warm-up during the input DMAs (ramps the clock to max) ----
    if n_warm > 0:
        warm_ps = psum.tile([128, 128], F32, tag="ps", name="warm_ps")
        for w in range(n_warm):
            nc.tensor.matmul(warm_ps[:], lhsT=tensors["gw1_sb"][:],
                             rhs=tensors["gw1_sb"][:],
                             start=(w == 0), stop=(w == n_warm - 1))
        warm_sb = pools["small"].tile([128, 1], F32, tag="warm_sb", name="warm_sb")
        nc.vector.tensor_copy(warm_sb[:], warm_ps[:, 0:1])
        warm_dr = pools["dram"].tile([128, 1], F32, tag="warm_dr", name="warm_dr")
        nc.sync.dma_start(warm_dr[:], warm_sb[:])

    # ---- conv (fp8 DoubleRow, taps {0,1} and {2,3-pad}) ----
    # h0T stores 32*h0 (conv_w was scaled x32 host-side; gW1 carries /32)
    h0T = acts.tile([128, BL, N], BF16, tag="hT", bufs=2, name="h0T")
    for nt in range(4):
        for s in range(BL):
            ps = psum.tile([128, 512], F32, tag="ps", name="ps_conv")
            for g in range(2):
                base = nt * 512 + 2 * g
                nc.tensor.matmul(
                    ps[:],
                    lhsT=wc_sb[:, g, :, :],
                    rhs=xT2_sb[:, s, :, base : base + 512],
                    start=(g == 0),
                    stop=(g == 1),
                    perf_mode=DoubleRow,
                )
            # fixups alternate ACT/DVE: one engine alone (0.7us/chunk)
            # would pace the consumer stage well below PE rate
            dst = h0T[:, s, nt * 512 : (nt + 1) * 512]
            if s % 2 == 0:
                nc.scalar.activation(dst, ps[:], Relu,
                                     bias=tensors["cb32_sb"][:])
            else:
                nc.vector.tensor_scalar(dst, ps[:], tensors["cb32_sb"][:],
                                        0.0, ALU_ADD, ALU_MAX)

    # ---- GCN layer 1 linear (bf16) -> hw1 fp8 node-major ----
    hw1 = acts.tile([128, 16, BL * 128], FP8, tag="hw8a", bufs=1, name="hw1")
    for nch in range(16):
        ps = psum.tile([128, 512], F32, tag="ps", name="ps_lin1")
        for s in range(BL):
            nc.tensor.matmul(
                ps[:, s * 128 : (s + 1) * 128],
                lhsT=h0T[:, s, nch * 128 : (nch + 1) * 128],
                rhs=tensors["gw1_sb"][:],
                start=True,
                stop=True,
            )
        if nch % 2 == 0:
            nc.vector.tensor_copy(hw1[:, nch, :], ps[:])
        else:
            nc.scalar.activation(hw1[:, nch, :], ps[:],
                                 mybir.ActivationFunctionType.Copy)

    # ---- GCN layer 1 aggregation (fp8 DoubleRow vs resident A8) ----
    h1T = acts.tile([128, BL, N], BF16, tag="hT", bufs=2, name="h1T")
    for dt in range(4):
        pss = [psum.tile([128, 512], F32, tag="ps", name=f"ps_agg1_{s}")
               for s in range(BL)]
        for sc2 in range(8):
            for s in range(BL):
                nc.tensor.matmul(
                    pss[s][:],
                    lhsT=hw1[:, 2 * sc2 : 2 * sc2 + 2, s * 128 : (s + 1) * 128],
                    rhs=at8_sb[:, 2 * sc2 : 2 * sc2 + 2, dt * 512 : (dt + 1) * 512],
                    start=(sc2 == 0),
                    stop=(sc2 == 7),
                    perf_mode=DoubleRow,
                )
        for s in range(BL):
            # relu(agg + gb1); GPSIMD can't touch PSUM, so split the 16
            # fixups between ACT and DVE to keep both under the PE time
            dst = h1T[:, s, dt * 512 : (dt + 1) * 512]
            if s < 2:
                nc.scalar.activation(dst, pss[s][:], Relu,
                                     bias=tensors["gb1_sb"][:])
            else:
                nc.vector.tensor_scalar(dst, pss[s][:],
                                        tensors["gb1_sb"][:], 0.0,
                                        ALU_ADD, ALU_MAX)

    # ---- GCN layer 2 linear (bf16, gW2 carries x16) -> hw2 fp8 ----
    hw2 = acts.tile([128, 16, BL * 128], FP8, tag="hw8b", bufs=1, name="hw2")
    for nch in range(16):
        ps = psum.tile([128, 512], F32, tag="ps", name="ps_lin2")
        for s in range(BL):
            nc.tensor.matmul(
                ps[:, s * 128 : (s + 1) * 128],
                lhsT=h1T[:, s, nch * 128 : (nch + 1) * 128],
                rhs=tensors["gw2_sb"][:],
                start=True,
                stop=True,
            )
        if nch % 2 == 0:
            nc.vector.tensor_copy(hw2[:, nch, :], ps[:])
        else:
            nc.scalar.activation(hw2[:, nch, :], ps[:],
                                 mybir.ActivationFunctionType.Copy)

    # ---- GCN layer 2 aggregation (fp8 DoubleRow), sample-outer so each
    #      sample's h2T finishes early for its AllToAll ----
    h2Ts = []
    for s in range(BL):
        # [128, 8 dest-core, 256] so the A2A staging is ONE DMA
        h2T_s = acts.tile([128, NCORES, NSH], BF16, tag=f"h2T{s}",
                          name=f"h2T{s}")
        pss = [psum.tile([128, 512], F32, tag="ps", name=f"ps_agg2_{dt}")
               for dt in range(4)]
        for sc2 in range(8):
            for dt in range(4):
                nc.tensor.matmul(
                    pss[dt][:],
                    lhsT=hw2[:, 2 * sc2 : 2 * sc2 + 2, s * 128 : (s + 1) * 128],
                    rhs=at8_sb[:, 2 * sc2 : 2 * sc2 + 2, dt * 512 : (dt + 1) * 512],
                    start=(sc2 == 0),
                    stop=(sc2 == 7),
                    perf_mode=DoubleRow,
                )
        for dt in range(4):
            # h2T stores 16*h2 (gW2 carried x16, gb2 prescaled x16;
            # W1 carries /16); ACT/DVE split halves the fixup drain that
            # gates this sample's AllToAll staging
            dst = h2T_s[:, 2 * dt : 2 * dt + 2, :]
            if dt < 2:
                nc.scalar.activation(dst, pss[dt][:], Relu,
                                     bias=tensors["gb2x16_sb"][:])
            else:
                nc.vector.tensor_scalar(dst, pss[dt][:],
                                        tensors["gb2x16_sb"][:], 0.0,
                                        ALU_ADD, ALU_MAX)
        h2Ts.append(h2T_s)

    return h2Ts


def _emit_tail(nc, tc, pools, tensors, h2Ts, out_ap, collectives=True):
    """AllToAll reshard + sharded MLP (z^T) + AllReduce + full local tail."""
    acts, psum, dram, small = (
        pools["acts"], pools["psum"], pools["dram"], pools["small"],
    )

    # One AllToAll per local sample, issued as soon as that sample's h2T
    # rows are done — staging + wire hide under the next sample's agg2.
    # Samples land in h2a column-order (s_local, core); the host unpermutes.
    h2a = acts.tile([128, BL, NCORES, NSH], BF16, tag="h2a", name="h2a")
    for s in range(BL):
        a2a_in = dram.tile([NCORES, 128, NSH], BF16, tag=f"a2a_in{s}",
                           name=f"a2a_in{s}")
        a2a_out = dram.tile([NCORES, 128, NSH], BF16, tag=f"a2a_out{s}",
                            name=f"a2a_out{s}")
        nc.sync.dma_start(a2a_in[0:4].rearrange("j p n -> p j n"),
                          h2Ts[s][:, 0:4, :])
        nc.sync.dma_start(a2a_in[4:8].rearrange("j p n -> p j n"),
                          h2Ts[s][:, 4:8, :])
        if collectives:
            nc.gpsimd.collective_compute(
                "AllToAll", mybir.AluOpType.bypass, replica_groups=RG,
                ins=[a2a_in.opt()], outs=[a2a_out.opt()],
            )
            nc.scalar.dma_start(h2a[:, s, :, :],
                                a2a_out.rearrange("i p n -> p i n"))
        else:
            # timing stand-in: skip the wire, read staged data directly
            nc.scalar.dma_start(h2a[:, s, :, :],
                                a2a_in.rearrange("i p n -> p i n"))

    # z^T[c, b] = sum_n w1[:, n, :].T @ h2a[:, :, :, n] — moving dim is
    # the samples, so each accumulating matmul costs only its row count.
    # Split into two column-halves: half 0 (local samples 0-1) starts as
    # soon as its two AllToAlls land, and its AllReduce roundtrip then
    # overlaps half 1's compute.
    ps_z = psum.tile([MLPD, 32], F32, tag="ps", name="ps_z")
    for h in range(2):
        for n in range(NSH):
            nc.tensor.matmul(
                ps_z[:, 16 * h : 16 * (h + 1)],
                lhsT=tensors["w1_sb"][:, n, :],
                rhs=h2a[:, 2 * h : 2 * h + 2, :, n],
                start=(n == 0),
                stop=(n == NSH - 1),
            )

    ar_in = dram.tile([MLPD, 32], F32, tag="ar_in", name="ar_in")
    ar_out = dram.tile([MLPD, 32], F32, tag="ar_out", name="ar_out")
    z_sb = small.tile([MLPD, 32], F32, tag="z_sb", name="z_sb")
    nc.vector.tensor_copy(z_sb[:], ps_z[:])
    nc.sync.dma_start(ar_in[:], z_sb[:])
    # hm has extra all-ones rows (memset partition base must be 32-aligned,
    # so fill 96:128 first and let the activation overwrite 0:100): the
    # final matmul against w2e (W2 + b2 + zero pad) lands b2 in ps_o.
    hm = small.tile([128, 32], F32, tag="hm", name="hm")
    nc.vector.memset(hm[96:128, :], 1.0)
    zar = small.tile([MLPD, 32], F32, tag="zar", name="zar")
    if collectives:
        nc.gpsimd.collective_compute(
            "AllReduce", mybir.AluOpType.add, replica_groups=RG,
            ins=[ar_in.opt()], outs=[ar_out.opt()],
        )
        nc.scalar.dma_start(zar[:], ar_out[:])
    else:
        nc.scalar.dma_start(zar[:], ar_in[:])

    nc.scalar.activation(hm[0:MLPD, :], zar[:], Relu, bias=tensors["b1c_sb"][:])
    ps_o = psum.tile([1, 32], F32, tag="ps", name="ps_o")
    nc.tensor.matmul(ps_o[:], lhsT=tensors["w2e_sb"][:], rhs=hm[:],
                     start=True, stop=True)
    osb = small.tile([1, 32], F32, tag="osb", name="osb")
    nc.vector.tensor_copy(osb[:], ps_o[:])
    nc.sync.dma_start(out_ap[:], osb[:])


def build_nc(front_reps=1, tail_reps=1, collectives=True, num_devices=NCORES,
             loop_all_reps=1, n_warm=16):
    """Build + compile the SPMD program. Reps>1 variants are for timing.

    loop_all_reps>1 wraps front+tail in a hardware loop with collectives
    replaced by equal-volume DMA stand-ins (collectives can't sit inside
    control flow) — used to measure whole-kernel steady-state time.
    """
    nc = bacc.Bacc("TRN2", target_bir_lowering=False, debug=False,
                   num_devices=num_devices)

    d_xT2 = nc.dram_tensor("xT2", [BL, 128, 2, HP], FP8, kind="ExternalInput").ap()
    d_at8 = nc.dram_tensor("at8", [16, 128, N], FP8, kind="ExternalInput").ap()
    d_wc = nc.dram_tensor("wc", [2, 128, 2, 128], FP8, kind="ExternalInput").ap()
    d_cb = nc.dram_tensor("cb32", [128, 1], F32, kind="ExternalInput").ap()
    d_gw1 = nc.dram_tensor("gw1", [128, 128], BF16, kind="ExternalInput").ap()
    d_gb1 = nc.dram_tensor("gb1", [128, 1], F32, kind="ExternalInput").ap()
    d_gw2 = nc.dram_tensor("gw2", [128, 128], BF16, kind="ExternalInput").ap()
    d_gb2 = nc.dram_tensor("gb2x16", [128, 1], F32, kind="ExternalInput").ap()
    d_w1s = nc.dram_tensor("w1s", [128, NSH, MLPD], BF16, kind="ExternalInput").ap()
    d_b1c = nc.dram_tensor("b1c", [MLPD, 1], F32, kind="ExternalInput").ap()
    d_w2e = nc.dram_tensor("w2e", [128, 1], F32, kind="ExternalInput").ap()
    d_out = nc.dram_tensor("out", [1, B], F32, kind="ExternalOutput").ap()

    with tile.TileContext(nc) as tc:
        with (
            tc.tile_pool(name="const", bufs=1) as const,
            tc.tile_pool(name="acts", bufs=1) as acts,
            tc.tile_pool(name="small", bufs=1) as small,
            tc.tile_pool(name="psum", bufs=8, space="PSUM") as psum,
            tc.tile_pool(name="dram", bufs=1, space="DRAM") as dram,
        ):
            pools = dict(const=const, acts=acts, small=small,
                         psum=psum, dram=dram)

            # ---- SP queue, in DMA-resource priority order: tiny conv
            #      weights, the x stream (conv is paced by it), then the
            #      four A^T quarters (agg1 dt=q can start after quarter q) ----
            gw1_sb = const.tile([128, 128], BF16, name="gw1_sb")
            nc.sync.dma_start(gw1_sb[:], d_gw1[:])
            wc_sb = const.tile([128, 2, 2, 128], FP8, name="wc_sb")
            nc.sync.dma_start(wc_sb[:], d_wc.rearrange("g p j o -> p g j o"))
            # x in half-sample chunks: conv nt-blocks 0-1 only need rows
            # <1026, so halving the DMA granularity halves conv's pacing
            # stalls at kernel start
            xT2_sb = const.tile([128, BL, 2, HP], FP8, name="xT2_sb")
            for s in range(BL):
                nc.sync.dma_start(xT2_sb[:, s, :, 0:1026], d_xT2[s][:, :, 0:1026])
            for s in range(BL):
                nc.sync.dma_start(xT2_sb[:, s, :, 1026:HP], d_xT2[s][:, :, 1026:HP])
            at8_sb = const.tile([128, 16, N], FP8, name="at8_sb")
            at8_dmas = []
            for q in range(4):
                at8_dmas.append(nc.sync.dma_start(
                    at8_sb[:, :, q * 512 : (q + 1) * 512],
                    d_at8[:, :, q * 512 : (q + 1) * 512]
                    .rearrange("c p d -> p c d"),
                ))

            # ---- Act queue: small weights (tiny transfers slot between
            #      the big SP ones) ----
            cb32_sb = const.tile([128, 1], F32, name="cb32_sb")
            nc.scalar.dma_start(cb32_sb[:], d_cb[:])
            gb1_sb = const.tile([128, 1], F32, name="gb1_sb")
            nc.scalar.dma_start(gb1_sb[:], d_gb1[:])
            gw2_sb = const.tile([128, 128], BF16, name="gw2_sb")
            nc.scalar.dma_start(gw2_sb[:], d_gw2[:])
            gb2x16_sb = const.tile([128, 1], F32, name="gb2x16_sb")
            nc.scalar.dma_start(gb2x16_sb[:], d_gb2[:])
            b1c_sb = small.tile([MLPD, 1], F32, name="b1c_sb")
            nc.scalar.dma_start(b1c_sb[:], d_b1c[:])
            w2e_sb = small.tile([128, 1], F32, name="w2e_sb")
            nc.scalar.dma_start(w2e_sb[:], d_w2e[:])

            # ---- gpsimd SWDGE: W1 streams on the otherwise-idle Pool
            #      queue, held back until A^T is in (the DMA engines are a
            #      shared resource; W1 isn't needed until the MLP ~40us in) ----
            w1_sb = const.tile([128, NSH, MLPD], BF16, name="w1_sb")
            for ch in range(4):
                w1_dma = nc.gpsimd.dma_start(
                    w1_sb[:, ch * 64 : (ch + 1) * 64, :],
                    d_w1s[:, ch * 64 : (ch + 1) * 64, :],
                )
                add_dep_helper(w1_dma.ins, at8_dmas[-1].ins,
                               reason="W1 stream must not starve x/A^T loads")

            tensors = dict(
                xT2_sb=xT2_sb, wc_sb=wc_sb, cb32_sb=cb32_sb, at8_sb=at8_sb,
                gw1_sb=gw1_sb, gb1_sb=gb1_sb, gw2_sb=gw2_sb,
                gb2x16_sb=gb2x16_sb, w1_sb=w1_sb, b1c_sb=b1c_sb,
                w2e_sb=w2e_sb,
            )

            if loop_all_reps > 1:
                with tc.For_i(0, loop_all_reps, 1,
                              hint_engines=(mybir.EngineType.PE,)):
                    h2Ts = _emit_front(nc, tc, pools, tensors, n_warm=0)
                    _emit_tail(nc, tc, pools, tensors, h2Ts, d_out,
                               collectives=False)
            elif front_reps == 1:
                h2Ts = _emit_front(nc, tc, pools, tensors, n_warm=n_warm)
                for _ in range(tail_reps):
                    _emit_tail(nc, tc, pools, tensors, h2Ts, d_out,
                               collectives=collectives)
            else:
                with tc.For_i(0, front_reps, 1,
                              hint_engines=(mybir.EngineType.PE,)):
                    h2Ts = _emit_front(nc, tc, pools, tensors, n_warm=0)
                for _ in range(tail_reps):
                    _emit_tail(nc, tc, pools, tensors, h2Ts, d_out,
                               collectives=collectives)

    nc.compile()
    return nc


def _prep_inputs(x, edge_index, conv_w, conv_b, gW1, gb1, gW2, gb2, W1, b1, W2, b2):
    """Host-side sharding / layout / scale-folding prep -> per-core inputs."""
    # gcn_norm (add_self_loops=True), duplicated edges accumulate
    src = np.concatenate([np.asarray(edge_index[0]), np.arange(N, dtype=np.int64)])
    dst = np.concatenate([np.asarray(edge_index[1]), np.arange(N, dtype=np.int64)])
    deg = np.bincount(dst, minlength=N).astype(np.float32)
    dinv = (1.0 / np.sqrt(np.maximum(deg, 1.0))).astype(np.float32)
    normv = dinv[src] * dinv[dst]
    AT = np.zeros((N, N), np.float32)
    np.add.at(AT, (src, dst), normv)
    at8_tiled = np.ascontiguousarray(AT.reshape(16, 128, N)).astype(NP_FP8)

    # conv weights: [C,1,KS,E] -> [g, ic, j, oc] with tap=2g+j, tap 3 zero,
    # prescaled x32 (fp8 normal range)
    wck = np.asarray(conv_w, np.float32)[:, 0, :, :].transpose(1, 2, 0)  # [KS, ic, oc]
    wc4 = np.zeros((2, 128, 2, 128), np.float32)
    for t in range(KS):
        wc4[t // 2, :, t % 2, :] = wck[t] * 32.0
    wc4 = wc4.astype(NP_FP8)
    cb32 = (np.asarray(conv_b, np.float32) * 32.0).reshape(128, 1)

    gw1 = (np.asarray(gW1, np.float32) / 32.0).astype(NP_BF16)
    gb1_ = np.asarray(gb1, np.float32).reshape(128, 1)
    gw2 = (np.asarray(gW2, np.float32) * 16.0).astype(NP_BF16)
    gb2x16 = (np.asarray(gb2, np.float32) * 16.0).reshape(128, 1)

    b1c = np.asarray(b1, np.float32).reshape(MLPD, 1)
    w2e = np.zeros((128, 1), np.float32)
    w2e[:MLPD, 0] = np.asarray(W2, np.float32)[:, 0]
    w2e[MLPD, 0] = np.asarray(b2, np.float32)[0]

    # W1 carries the /16 that un-scales h2
    W1r = (np.asarray(W1, np.float32) / 16.0).reshape(N, G2, MLPD)

    # x duplicated at row offsets {0,1}, padded, fp8
    x_np = np.asarray(x, np.float32)
    xpad = np.zeros((B, HP + 1, E), np.float32)
    xpad[:, :H, :] = x_np

    in_maps = []
    for c in range(NCORES):
        xT2 = np.empty((BL, 128, 2, HP), np.float32)
        for j in range(2):
            xT2[:, :, j, :] = xpad[c * BL : (c + 1) * BL, j : j + HP].transpose(0, 2, 1)
        w1s = np.ascontiguousarray(
            W1r[c * NSH : (c + 1) * NSH].transpose(1, 0, 2)
        ).astype(NP_BF16)
        in_maps.append({
            "xT2": xT2.astype(NP_FP8), "at8": at8_tiled, "wc": wc4,
            "cb32": cb32, "gw1": gw1, "gb1": gb1_, "gw2": gw2,
            "gb2x16": gb2x16, "w1s": w1s, "b1c": b1c, "w2e": w2e,
        })
    return in_maps


_NC_CACHE = {}


def kernel(**inputs) -> np.ndarray:
    key = "full"
    if key not in _NC_CACHE:
        _NC_CACHE[key] = build_nc()
    nc = _NC_CACHE[key]
    in_maps = _prep_inputs(**inputs)
    res = run_bass_kernel_spmd(nc, in_maps, core_ids=list(range(NCORES)))
    # z columns are ordered (s_local, core): column s*8+i is global sample i*4+s
    out_perm = np.asarray(res.results[0]["out"], np.float32).reshape(BL, NCORES)
    out = out_perm.T.reshape(B, 1)
    return np.ascontiguousarray(out)
